# revision 21
# baseline (speedup 1.0000x reference)
"""Llama MHA (B=2, S=2048, D=2048, H=16, causal, RoPE) on 8 trn2 cores.

Sharding: data-parallel over batch (2 groups of 4 cores) x tensor-parallel
over heads (4 heads per core). Each core computes, for its (batch, 4 heads):
  qT/kT = w^T x^T  (features on partitions, seq on free dim)
  RoPE on qT/kT (weights column-permuted on host so even/odd feature pairs
  land de-interleaved: rows 0:64 = even, 64:128 = odd; dot products are
  permutation-invariant so scores match the reference exactly)
  scoresT[k,q] blocks -> exp (no max subtraction needed: |score*scale| <~ 6)
  -> causal mask on diagonal blocks -> PV matmuls; causal column trimming
  on diagonal blocks (block j only feeds queries q_local >= 128j).
  Softmax denominator: exp'd blocks are accumulated in-place on the DVE
  (bf16) and reduced across keys with a single all-ones matmul per
  (chunk, head) - every PSUM row = key-sum, so the broadcast divide is free.
  -> normalize -> out projection partial resT = wo^T attnT.
Host sums the 4 partials per batch and transposes back.

Schedule: per seq-chunk sc, emit proj(sc) -> attn(qc=sc) -> enqueue
outproj(sc). Out-projection steps are pumped as PE filler inside later
attention chains so the PE never idles waiting for the Scalar engine's
exp (exp at ~543ns/block vs score+PV at ~432ns/block).

All matmuls in bf16 (fp32 PSUM accumulation); softmax/normalization in fp32.
"""

import numpy as np
import ml_dtypes

import concourse.bass as bass
import concourse.mybir as mybir
import concourse.tile as tile
from concourse import bacc
from concourse.bass_utils import run_bass_kernel_spmd

B, S, D, H = 2, 2048, 2048, 16
DH = D // H            # 128 head dim
HPC = 4                # heads per core
N_CORES = 8
FH = HPC * DH          # 512 features per core
P = 128
KT = D // P            # 16 k-tiles over D
SC = S // 512          # 4 seq chunks of 512
ST = S // P            # 16 seq blocks of 128
THETA = 10000.0
SCALE = 1.0 / np.sqrt(DH)

DT = mybir.dt.bfloat16
NPDT = ml_dtypes.bfloat16

_prog_cache = {}


def _build():
    if "nc" in _prog_cache:
        return _prog_cache["nc"]
    nc = bacc.Bacc(None, target_bir_lowering=False, debug=False)

    xT = nc.dram_tensor("xT", [D, S], DT, kind="ExternalInput")
    wq = nc.dram_tensor("wq", [D, FH], DT, kind="ExternalInput")
    wk = nc.dram_tensor("wk", [D, FH], DT, kind="ExternalInput")
    wv = nc.dram_tensor("wv", [D, FH], DT, kind="ExternalInput")
    wo = nc.dram_tensor("wo", [FH, D], DT, kind="ExternalInput")
    cc = nc.dram_tensor("cc", [P, S], DT, kind="ExternalInput")
    ss = nc.dram_tensor("ss", [P, S], DT, kind="ExternalInput")
    masks = nc.dram_tensor("masks", [P, 4, 512], DT, kind="ExternalInput")
    resT = nc.dram_tensor("resT", [D, S], DT, kind="ExternalOutput")

    f32 = mybir.dt.float32

    with tile.TileContext(nc) as tc:
        with (
            tc.tile_pool(name="persist", bufs=1) as pp,
            tc.tile_pool(name="psA", bufs=3, space="PSUM") as psA,
            tc.tile_pool(name="psO", bufs=2, space="PSUM") as psO,
            tc.tile_pool(name="psD", bufs=1, space="PSUM") as psD,
            tc.tile_pool(name="psP", bufs=2, space="PSUM") as psP,
            tc.tile_pool(name="wpool", bufs=1) as wp,
            tc.tile_pool(name="xpool", bufs=2) as xp,
            tc.tile_pool(name="ropetmp", bufs=1) as rp,
            tc.tile_pool(name="ppool", bufs=6) as ptp,
            tc.tile_pool(name="npool", bufs=2) as np_,
            tc.tile_pool(name="dpool", bufs=2) as dp,
            tc.tile_pool(name="rpool", bufs=3) as rop,
        ):
            qT = pp.tile([P, HPC, S], DT)     # per head: rows=feat, free=seq
            kT = pp.tile([P, HPC, S], DT)
            vn = pp.tile([P, ST, FH], DT)     # v natural: [seq-block, feat]
            attnT = pp.tile([P, HPC, S], DT)  # normalized attention output^T
            # cos/sin tables split per 512-chunk: separate tiles so the
            # first RoPE doesn't wait (whole-tile dep) on late chunk DMAs
            cc_t = [pp.tile([P, 512], DT, name=f"cct{i}") for i in range(SC)]
            ss_t = [pp.tile([P, 512], DT, name=f"sst{i}") for i in range(SC)]
            mask_t = pp.tile([P, 4, 512], DT)
            ones_mat = pp.tile([P, P], DT)    # denominator stationary: the
                                              # [128,128] all-ones matrix makes
                                              # every PSUM row the key-sum, so
                                              # the broadcast is free
            nc.vector.memset(ones_mat, 1.0)
            wo_t = pp.tile([P, HPC, D], DT)

            G4 = (2, 2, 4, 8)
            G4_OFF = (0, 2, 4, 8)
            wqg = [wp.tile([P, n, FH], DT, name=f"wqg{i}")
                   for i, n in enumerate(G4)]
            xcg = [wp.tile([P, n, 512], DT, name=f"xcg{i}")
                   for i, n in enumerate(G4)]
            wk_t = wp.tile([P, KT, FH], DT)
            wv_t = wp.tile([P, KT, FH], DT)

            def g4idx(gtiles, k):
                for gi in range(len(G4) - 1, -1, -1):
                    if k >= G4_OFF[gi]:
                        return gtiles[gi][:, k - G4_OFF[gi], :]
                raise AssertionError

            wqr = wq.rearrange("(kt p) f -> p kt f", p=P)
            xr = xT.rearrange("(kt p) s -> p kt s", p=P)
            # DMA issue order is the Sync-queue order; critical-path tensors
            # go strictly first. Each group is one DMA into its own tile so
            # the first q chain's k-tile deps resolve incrementally.
            for gi, n in enumerate(G4):
                gs = slice(G4_OFF[gi], G4_OFF[gi] + n)
                nc.sync.dma_start(out=wqg[gi], in_=wqr[:, gs, :])
                nc.sync.dma_start(out=xcg[gi], in_=xr[:, gs, 0:512])
            nc.sync.dma_start(out=cc_t[0], in_=cc[:, 0:512])
            nc.sync.dma_start(out=ss_t[0], in_=ss[:, 0:512])
            wkr = wk.rearrange("(kt p) f -> p kt f", p=P)
            for g in range(4):
                gs = slice(g * 4, (g + 1) * 4)
                nc.sync.dma_start(out=wk_t[:, gs, :], in_=wkr[:, gs, :])
            nc.sync.dma_start(out=wv_t, in_=wv.rearrange("(kt p) f -> p kt f", p=P))
            for i in range(1, SC):
                nc.sync.dma_start(out=cc_t[i], in_=cc[:, i * 512:(i + 1) * 512])
                nc.sync.dma_start(out=ss_t[i], in_=ss[:, i * 512:(i + 1) * 512])
            nc.sync.dma_start(out=mask_t, in_=masks[:, :, :])

            # -------- out-projection steps, pumped as PE filler ----------
            filler = []

            def pump(n):
                for _ in range(min(n, len(filler))):
                    filler.pop(0)()

            def enqueue_outproj(sc):
                csl = slice(sc * 512, (sc + 1) * 512)
                for db in range(KT):
                    def step(db=db, csl=csl, sc=sc):
                        pr = psP.tile([P, 512], f32, tag="pr",
                                      name=f"pr{sc}{db}")
                        for ft in range(HPC):
                            nc.tensor.matmul(
                                pr, wo_t[:, ft, db * P:(db + 1) * P],
                                attnT[:, ft, csl],
                                start=(ft == 0), stop=(ft == HPC - 1),
                            )
                        rt = rop.tile([P, 512], DT, tag="rt")
                        nc.vector.tensor_copy(rt, pr)
                        nc.sync.dma_start(
                            out=resT[db * P:(db + 1) * P, csl], in_=rt)
                    filler.append(step)

            xcs = [None] + [
                xp.tile([P, KT, 512], DT, tag="xc", name=f"xc{i}")
                for i in range(1, SC)
            ]
            for sc in range(SC):
                # ---------------- projections + RoPE ---------------------
                xc = xcs[sc]
                csl = slice(sc * 512, (sc + 1) * 512)
                # q/k projections with RoPE fused into the PSUM drain.
                for wt, dst in (("q", qT), ("k", kT)):
                    for h in range(HPC):
                        fsl = slice(h * DH, (h + 1) * DH)
                        pq = psA.tile([P, 512], f32, tag="ps", name=f"pq{sc}{h}")
                        for k in range(KT):
                            wap = (g4idx(wqg, k)[:, fsl] if wt == "q"
                                   else wk_t[:, k, fsl])
                            xap = (g4idx(xcg, k) if sc == 0 else xc[:, k, :])
                            nc.tensor.matmul(
                                pq, wap, xap,
                                start=(k == 0), stop=(k == KT - 1),
                            )
                        # RoPE: dst = pq*cc + swap(pq)*(+/-ss)
                        # ss_t rows 0:64 = +sin (feeds bottom), rows
                        # 64:128 = -sin (feeds top); swap is done by
                        # writing each product into the opposite half
                        # so every DVE op has aligned base partitions.
                        ta = rp.tile([P, 512], f32, tag="ta")
                        tb = rp.tile([P, 512], f32, tag="tb")
                        nc.vector.tensor_mul(ta, pq, cc_t[sc])
                        nc.vector.tensor_mul(
                            tb[0:64, :], pq[64:128, :], ss_t[sc][64:128, :])
                        nc.vector.tensor_mul(
                            tb[64:128, :], pq[0:64, :], ss_t[sc][0:64, :])
                        nc.vector.tensor_add(dst[:, h, csl], ta, tb)
                # v projection straight into natural layout
                for st4 in range(4):
                    sb = sc * 4 + st4
                    pv = psA.tile([P, FH], f32, tag="ps", name=f"pv{sc}{st4}")
                    for k in range(KT):
                        xap = (g4idx(xcg, k) if sc == 0 else xc[:, k, :])
                        nc.tensor.matmul(
                            pv, xap[:, st4 * P:(st4 + 1) * P], wv_t[:, k, :],
                            start=(k == 0), stop=(k == KT - 1),
                        )
                    nc.vector.tensor_copy(vn[:, sb, :], pv)

                # prefetch next x chunk (+wo) before attention's out-DMAs
                # and pumped filler occupy the Sync queue
                if sc + 1 < SC:
                    nxt = xcs[sc + 1]
                    for g in range(4):
                        gs = slice(g * 4, (g + 1) * 4)
                        nc.sync.dma_start(
                            out=nxt[:, gs, :],
                            in_=xr[:, gs, (sc + 1) * 512:(sc + 2) * 512])
                if sc == 1:
                    nc.sync.dma_start(
                        out=wo_t, in_=wo.rearrange("(ft p) d -> p ft d", p=P))

                # ---------------- attention for qc = sc ------------------
                qc = sc
                qsl = slice(qc * 512, (qc + 1) * 512)
                for h in range(HPC):
                    fsl = slice(h * DH, (h + 1) * DH)
                    po = psO.tile([P, 512], f32, tag="po", name=f"po{h}{qc}")
                    pd = psD.tile([P, 512], f32, tag="pd", name=f"pd{h}{qc}")
                    dsum = dp.tile([P, 512], DT, tag="ds", name=f"ds{h}{qc}")
                    nkb = 4 * qc + 4
                    pt0 = None
                    for kb in range(nkb):
                        # causal column trim: diagonal key-block j only
                        # affects queries q_local >= 128*j
                        j = kb - 4 * qc
                        c0 = max(0, j * P)
                        cw = slice(c0, 512)
                        ps = psA.tile([P, 512], f32, tag="ps",
                                      name=f"ps{h}{qc}{kb}")
                        nc.tensor.matmul(
                            ps[:, cw], kT[:, h, kb * P:(kb + 1) * P],
                            qT[:, h, qc * 512 + c0:(qc + 1) * 512],
                            start=True, stop=True,
                        )
                        pt = ptp.tile([P, 512], DT, tag="pt")
                        nc.scalar.activation(
                            pt[:, cw], ps[:, cw],
                            mybir.ActivationFunctionType.Exp,
                            scale=float(SCALE),
                        )
                        if j >= 0:
                            nc.vector.tensor_mul(
                                pt[:, cw], pt[:, cw], mask_t[:, j, cw])
                        # denominator accumulation (bf16, in-place on DVE)
                        if kb == 0:
                            pt0 = pt
                        elif kb == 1:
                            c1 = max(0, (1 - 4 * qc) * P)
                            nc.vector.tensor_add(
                                dsum[:, c1:], pt0[:, c1:], pt[:, c1:])
                            if qc == 0:
                                nc.vector.tensor_copy(
                                    dsum[:, 0:P], pt0[:, 0:P])
                        else:
                            nc.vector.tensor_add(
                                dsum[:, cw], dsum[:, cw], pt[:, cw])
                        nc.tensor.matmul(
                            po[:, cw], vn[:, kb, fsl], pt[:, cw],
                            start=(kb == 0), stop=(kb == nkb - 1),
                            skip_group_check=True,
                        )
                        if kb % 2 == 1:
                            pump(1)
                    # single key-sum reduction for the whole chain
                    nc.tensor.matmul(pd, ones_mat, dsum, start=True, stop=True)
                    bc = np_.tile([P, 512], f32, tag="bc")
                    nc.vector.reciprocal_approx_fast(out=bc, in_=pd)
                    nc.vector.tensor_mul(attnT[:, h, qsl], po, bc)
                enqueue_outproj(sc)

            pump(len(filler))

    nc.finalize()
    _prog_cache["nc"] = nc
    return nc


def _host_inputs(x, w_q, w_k, w_v, w_o):
    """Build the 8 per-core input maps."""
    # RoPE de-interleave permutation per head: evens then odds
    i = np.arange(DH)
    perm_head = np.concatenate([i[0::2], i[1::2]])  # within-head column order

    t = np.arange(S, dtype=np.float64)
    inv_freq = 1.0 / (THETA ** (np.arange(0, DH, 2, dtype=np.float64) / DH))
    ang = np.outer(t, inv_freq)          # [S, 64]
    cosT = np.cos(ang).T.astype(np.float32)   # [64, S]
    sinT = np.sin(ang).T.astype(np.float32)
    cc = np.vstack([cosT, cosT])         # [128, S]
    ss = np.vstack([sinT, -sinT])        # +sin feeds bottom half, -sin top

    # diagonal causal masks: mask[j][k, q] = 1 if 128*j + k <= q
    kk = np.arange(P)[:, None]
    qq = np.arange(512)[None, :]
    masks = np.stack(
        [(P * j + kk <= qq) for j in range(4)], axis=1
    ).astype(NPDT)                        # [128, 4, 512]

    in_maps = []
    for core in range(N_CORES):
        b = core // 4
        h0 = (core % 4) * HPC
        cols = np.concatenate(
            [h * DH + perm_head for h in range(h0, h0 + HPC)])   # rope-permuted
        vcols = np.arange(h0 * DH, (h0 + HPC) * DH)              # natural
        in_maps.append({
            "xT": np.ascontiguousarray(x[b].T).astype(NPDT),
            "wq": np.ascontiguousarray(w_q[:, cols]).astype(NPDT),
            "wk": np.ascontiguousarray(w_k[:, cols]).astype(NPDT),
            "wv": np.ascontiguousarray(w_v[:, vcols]).astype(NPDT),
            "wo": np.ascontiguousarray(w_o[vcols, :]).astype(NPDT),
            "cc": cc.astype(NPDT),
            "ss": ss.astype(NPDT),
            "masks": masks,
        })
    return in_maps


def kernel(x, w_q, w_k, w_v, w_o, _trace=False, _results_out=None):
    x = np.asarray(x, dtype=np.float32)
    w_q = np.asarray(w_q, dtype=np.float32)
    w_k = np.asarray(w_k, dtype=np.float32)
    w_v = np.asarray(w_v, dtype=np.float32)
    w_o = np.asarray(w_o, dtype=np.float32)
    nc = _build()
    in_maps = _host_inputs(x, w_q, w_k, w_v, w_o)
    res = run_bass_kernel_spmd(
        nc, in_maps, core_ids=list(range(N_CORES)), trace=_trace)
    if _results_out is not None:
        _results_out.append(res)
    out = np.empty((B, S, D), np.float32)
    for b in range(B):
        acc = res.results[4 * b]["resT"].astype(np.float32)
        for g in range(1, 4):
            acc = acc + res.results[4 * b + g]["resT"]
        out[b] = acc.T
    return out
